# revision 1
# baseline (speedup 1.0000x reference)
"""Trainium2 Bass kernel for Conv2dAffine8bit.

Reference computation:
    w_dq = affine-uint8 quantize-dequantize(weight)   (per-tensor scale/zp)
    out  = conv2d(x, w_dq, stride 1, pad 1) + bias    (NCHW, OIHW)
with x [32, 256, 56, 56] f32, weight [256, 256, 3, 3] f32, bias [256] f32.

Strategy (8 NeuronCores, data-parallel over batch, 4 images per core):
  * Host reproduces the quantization math bit-exactly in fp32. The integer
    weights (w_q - zero_point, in [-255, 255]) are EXACT in bf16/f32r, so the
    conv runs against integer weights and `scale` folds into the epilogue
    (conv is linear in w).
  * conv = 9 shifted matmuls (3x3 taps) over a zero-padded [58, 58] image in
    SBUF: out[co, r, :] accumulates lhsT[ci, co] @ x[ci, r+ky, kx:kx+56] into
    PSUM tiles of [128 cout, 8 rows x 56 cols = 448].
  * Two precision modes:
      - "f32r" (default): PE's 12-bit-significand fp32 mode, full bf16-rate
        for free dim >= 256. 9 taps x 2 cin-halves = 18 matmuls per PSUM
        tile; ~1e-4 relative error.
      - "hilo": x split into bf16 hi + lo (x = hi + lo keeps ~16 mantissa
        bits); 36 matmuls per PSUM tile; ~3e-6 relative error, 2x the PE time.
  * Epilogue: ScalarE Identity activation computes psum * scale + bias[co],
    then DMA to the output shard.
"""

import os

import numpy as np
import ml_dtypes

import concourse.tile as tile
from concourse import bacc, mybir
from concourse.bass_utils import run_bass_kernel_spmd

N_CORES = 8
N_IMGS = 32
IMGS_PER_CORE = N_IMGS // N_CORES
C = 256  # in and out channels
H = W = 56
HP = WP = 58  # padded
R = 8  # output rows per PSUM block
N_BLOCKS = H // R  # 7
FREE = R * W  # 448

MODE = os.environ.get("CONV_MODE", "f32r")  # "f32r" | "hilo"

_BF16 = ml_dtypes.bfloat16

_cache: dict = {}


def _build(scale: float, mode: str):
    """Build + compile the per-core Bass program. `scale` is baked as an
    immediate in the epilogue, so cache on it."""
    key = (scale, mode)
    if key in _cache:
        return _cache[key]

    nc = bacc.Bacc()
    dt = mybir.dt
    x_dt = dt.float32r if mode == "f32r" else dt.bfloat16
    w_dt = x_dt
    if mode == "f32r":
        d_xs = [nc.declare_dram_parameter(
            "x0", [IMGS_PER_CORE, C, HP, WP], x_dt, isOutput=False)]
        parts = ("x0",)
    else:
        d_xs = [
            nc.declare_dram_parameter(
                "xhi", [IMGS_PER_CORE, C, HP, WP], x_dt, isOutput=False),
            nc.declare_dram_parameter(
                "xlo", [IMGS_PER_CORE, C, HP, WP], x_dt, isOutput=False),
        ]
        parts = ("xhi", "xlo")
    # wq[ci, ci_half, tap, co_half, co] = (w_q - zp)[co_half*128+co, ci_half*128+ci, tap]
    d_wq = nc.declare_dram_parameter("wq", [128, 2, 9, 2, 128], w_dt, isOutput=False)
    d_bias = nc.declare_dram_parameter("bias", [128, 2], dt.float32, isOutput=False)
    d_y = nc.declare_dram_parameter(
        "y", [IMGS_PER_CORE, C, H, W], dt.float32, isOutput=True)

    with tile.TileContext(nc) as tc:
        with (
            tc.tile_pool(name="wpool", bufs=1) as wpool,
            tc.tile_pool(name="xpool", bufs=3) as xpool,
            tc.tile_pool(name="opool", bufs=4) as opool,
            tc.tile_pool(name="pspool", bufs=4, space="PSUM") as pspool,
        ):
            t_wq = wpool.tile([128, 2, 9, 2, 128], w_dt, tag="wq")
            nc.sync.dma_start(t_wq[:], d_wq[:])
            t_bias = wpool.tile([128, 2], dt.float32, tag="bias")
            nc.sync.dma_start(t_bias[:], d_bias[:])

            n_mm = 18 * len(parts)
            for img in range(IMGS_PER_CORE):
                # x tiles for this image: [128 ci, 58, 58] per (half, part)
                xt = {}
                for ci_half in range(2):
                    for part, src in zip(parts, d_xs):
                        t = xpool.tile([128, HP, WP], x_dt,
                                       tag=f"x_{part}{ci_half}")
                        nc.sync.dma_start(
                            t[:], src[img, ci_half * 128:(ci_half + 1) * 128])
                        xt[(ci_half, part)] = t

                for co_half in range(2):
                    for blk in range(N_BLOCKS):
                        r0 = blk * R
                        ps = pspool.tile([128, FREE], dt.float32, tag="ps")
                        i_mm = 0
                        for ky in (0, -1, 1):
                            for kx in (-1, 0, 1):
                                tap = (ky + 1) * 3 + (kx + 1)
                                for ci_half in range(2):
                                    lhsT = t_wq[:, ci_half, tap, co_half, :]
                                    for part in parts:
                                        rhs = xt[(ci_half, part)][
                                            :, r0 + ky + 1: r0 + ky + 1 + R,
                                            kx + 1: kx + 1 + W]
                                        nc.tensor.matmul(
                                            ps[:], lhsT, rhs,
                                            start=(i_mm == 0),
                                            stop=(i_mm == n_mm - 1))
                                        i_mm += 1
                        ob = opool.tile([128, FREE], dt.float32, tag="ob")
                        nc.scalar.activation(
                            ob[:], ps[:], mybir.ActivationFunctionType.Identity,
                            bias=t_bias[:, co_half:co_half + 1], scale=float(scale))
                        nc.scalar.dma_start(
                            d_y[img, co_half * 128:(co_half + 1) * 128,
                                r0:r0 + R, :],
                            ob[:].rearrange("p (r c) -> p r c", c=W))

    nc.compile()
    _cache[key] = nc
    return nc


def _quantize_weight(weight: np.ndarray):
    """Bit-exact fp32 replica of the reference affine-uint8 quantization.
    Returns (w_int, scale): w_int = w_q - zero_point (integers in [-255, 255],
    exact in bf16) and the per-tensor fp32 scale."""
    w = np.asarray(weight, dtype=np.float32)
    min_val = np.min(w)
    max_val = np.max(w)
    scale = np.float32(np.float32(max_val - min_val) / np.float32(255.0))
    zp = np.round(np.clip(np.float32(255.0) - np.float32(max_val / scale),
                          np.float32(0.0), np.float32(255.0)))
    w_q = np.round(np.clip(zp + w / scale, np.float32(0.0), np.float32(255.0)))
    w_int = (w_q - zp).astype(np.float32)
    return w_int, scale


def kernel(x, weight, bias):
    x = np.asarray(x, dtype=np.float32)
    weight = np.asarray(weight, dtype=np.float32)
    bias = np.asarray(bias, dtype=np.float32)

    w_int, scale = _quantize_weight(weight)

    # lhsT layout [ci, ci_half, tap, co_half, co]
    w_r = w_int.reshape(2, 128, 2, 128, 9)  # [co_half, co, ci_half, ci, tap]
    wq_host = np.ascontiguousarray(np.transpose(w_r, (3, 2, 4, 0, 1)))
    bias_host = np.ascontiguousarray(bias.reshape(2, 128).T)  # [128, 2]

    # pad to [N, C, 58, 58]
    xp = np.zeros((N_IMGS, C, HP, WP), dtype=np.float32)
    xp[:, :, 1:1 + H, 1:1 + W] = x

    if MODE == "f32r":
        x_parts = {"x0": xp}  # raw f32 bits; PE rounds on ingest
        wq_host = wq_host.astype(np.float32)
    else:
        x_hi = xp.astype(_BF16)
        x_lo = (xp - x_hi.astype(np.float32)).astype(_BF16)
        x_parts = {"xhi": x_hi, "xlo": x_lo}
        wq_host = wq_host.astype(_BF16)

    nc = _build(float(scale), MODE)
    in_maps = []
    for c in range(N_CORES):
        sl = slice(c * IMGS_PER_CORE, (c + 1) * IMGS_PER_CORE)
        m = {name: arr[sl] for name, arr in x_parts.items()}
        m["wq"] = wq_host
        m["bias"] = bias_host
        in_maps.append(m)
    res = run_bass_kernel_spmd(nc, in_maps, list(range(N_CORES)))
    return np.concatenate([res.results[c]["y"] for c in range(N_CORES)], axis=0)

